# revision 1
# baseline (speedup 1.0000x reference)
"""DCN cross-network forward on 8 Trainium2 NeuronCores.

Reference computation (LAYER_NUM=4, INPUT_DIM=1024, BATCH=16384):
    x0 = x
    for i in range(4):
        s  = xi @ w[i]                      # [B] per-row scalar
        xi = x0 * s[:, None] + b[i] + xi

Algebraic collapse used here: every layer adds a per-row multiple of x0
plus a constant vector, so
    x_i = alpha_i * x0 + C_i,   C_i = sum_{j<i} b[j]          (constant vec)
    t_i = x0 . w[i]             (per-row scalars)
    k_i = C_i . w[i]            (host-computable scalar constants)
    alpha_0 = 1,  alpha_{i+1} = alpha_i * (1 + t_i) + k_i
    out = alpha_4 * x0 + C_4
which reads x exactly once and writes out exactly once (memory roofline).
The C_4 broadcast-add runs on the host (numpy) — zero device time.

Implementation notes:
  - x and w are extended with a constant 1.0 column (DE = 1025) so every
    dot product returns 1 + t_i directly — the recurrence then needs
    only 3 fused ops per tile.
  - PE tiles (odd j): dots via TensorE transpose + matmul (+1 via an
    extra rank-1 ones matmul in the accumulation group); recurrence on
    DVE; out-scale on ScalarE.
  - Vector tiles (even j): dots 0,1 fused on DVE (scalar_tensor_tensor
    with accum_out — tensor_tensor_reduce faults on TRN2 hw); dots 2,3
    as GpSimd multiply + ScalarE activation-accumulate; recurrence on
    ScalarE via chained activations; out-scale on DVE
    (tensor_scalar_mul, single-source 2x mode).

Sharding: data-parallel over batch; each of the 8 cores processes a
[2048, 1024] slice with replicated small weights.
"""

import sys

import numpy as np

sys.path.insert(0, "/opt/trn_rl_repo")

BATCH = 16384
D = 1024
DE = D + 1               # x/w extended with a ones column
L = 4
NCORES = 8
SHARD = BATCH // NCORES  # 2048
P = 128
NT = SHARD // P          # 16 row-tiles per core
NCH = D // P             # 8 contraction chunks

# tiles whose dot products run on the TensorEngine (transpose+matmul)
PE_TILES = frozenset({0, 2, 4, 5, 6, 8, 10, 12, 14})

_build_cache: dict = {}


def _build_program(k1: float, k2: float, k3: float):
    """Build (and compile) the SPMD Bass program for one core's shard."""
    import concourse.bacc as bacc
    import concourse.mybir as mybir
    import concourse.tile as tile
    f32 = mybir.dt.float32
    mult = mybir.AluOpType.mult
    add = mybir.AluOpType.add
    Copy = mybir.ActivationFunctionType.Copy

    nc = bacc.Bacc("TRN2", target_bir_lowering=False, debug=False)

    x = nc.dram_tensor("x", [SHARD, DE], f32, kind="ExternalInput").ap()
    wbd = nc.dram_tensor("wbd", [P, L, DE], f32, kind="ExternalInput").ap()
    wtd = nc.dram_tensor("wtd", [P, NCH, L], f32, kind="ExternalInput").ap()
    idd = nc.dram_tensor("idd", [P, P], f32, kind="ExternalInput").ap()
    out = nc.dram_tensor("out", [SHARD, D], f32, kind="ExternalOutput").ap()

    with tile.TileContext(nc) as tc:
        with (
            tc.tile_pool(name="consts", bufs=1) as cpool,
            tc.tile_pool(name="xin", bufs=4) as xpool,
            tc.tile_pool(name="xtr", bufs=3) as xtpool,
            tc.tile_pool(name="scr", bufs=6) as scrpool,
            tc.tile_pool(name="small", bufs=6) as spool,
            tc.tile_pool(name="outp", bufs=3) as opool,
            tc.tile_pool(name="ps_tr", bufs=3, space="PSUM") as pst,
            tc.tile_pool(name="ps_t", bufs=2, space="PSUM") as psv,
        ):
            ident = cpool.tile([P, P], f32)
            with tc.high_priority():
                nc.sync.dma_start(out=ident[:], in_=idd)
            ones1 = cpool.tile([1, P], f32)
            nc.vector.memset(ones1[:], 1.0)
            ones4 = cpool.tile([1, L], f32)
            nc.vector.memset(ones4[:], 1.0)
            # w^T chunks for the PE dot path: wt_sb[p, c, i] = w[i, c*128+p]
            wt_sb = cpool.tile([P, NCH, L], f32)
            with tc.high_priority():
                nc.sync.dma_start(out=wt_sb[:], in_=wtd)
            # w rows replicated across partitions (pre-broadcast on host)
            wb = cpool.tile([P, L, DE], f32)
            with tc.high_priority():
                for i in range(L):
                    nc.sync.dma_start(out=wb[:, i, :], in_=wbd[:, i, :])

            xr = x.rearrange("(s h p) d -> s p h d", p=P, h=4)
            outr = out.rearrange("(s h p) d -> s p h d", p=P, h=4)
            xt2 = None
            o2 = None
            for j in range(NT):
                s_idx, h = j // 4, j % 4
                if h == 0:
                    xt2 = xpool.tile([P, 4, DE], f32, tag="x")
                    with tc.high_priority(offset=15):
                        if s_idx <= 1:
                            # fine-grained first group: start compute sooner
                            for hh in range(4):
                                nc.sync.dma_start(
                                    out=xt2[:, hh, :], in_=xr[s_idx, :, hh, :]
                                )
                        else:
                            nc.sync.dma_start(out=xt2[:], in_=xr[s_idx])
                    o2 = opool.tile([P, 4, D], f32, tag="o")
                xt = xt2[:, h, :]

                if j in PE_TILES:
                    # --- PE path: transpose chunks, matmul against w^T ---
                    xtp = pst.tile([P, D], f32, tag="xtp")
                    for c in range(NCH):
                        nc.tensor.transpose(
                            xtp[:, c * P : (c + 1) * P],
                            xt[:, c * P : (c + 1) * P],
                            ident[:],
                        )
                    xts = xtpool.tile([P, D], f32, tag="xts")
                    nc.scalar.copy(out=xts[:, : D * 5 // 8], in_=xtp[:, : D * 5 // 8])
                    nc.vector.tensor_copy(xts[:, D * 5 // 8 :], xtp[:, D * 5 // 8 :])
                    tps = psv.tile([P, L], f32, tag="tps")
                    for c in range(NCH):
                        nc.tensor.matmul(
                            tps[:],
                            lhsT=xts[:, c * P : (c + 1) * P],
                            rhs=wt_sb[:, c, :],
                            start=(c == 0),
                            stop=False,
                        )
                    # += 1 everywhere: rank-1 ones update closes the group
                    nc.tensor.matmul(
                        tps[:], lhsT=ones1[:], rhs=ones4[:],
                        start=False, stop=True,
                    )
                    # --- recurrence on DVE (copy to SBUF, 3 fused ops) ---
                    tvp = spool.tile([P, L], f32, tag="tvp")
                    nc.vector.tensor_copy(tvp[:], tps[:])
                    a2 = spool.tile([P, 1], f32, tag="a2")
                    nc.vector.tensor_scalar(
                        a2[:], tvp[:, 0:1], tvp[:, 1:2], k1, op0=mult, op1=add
                    )
                    a3 = spool.tile([P, 1], f32, tag="a3")
                    nc.vector.tensor_scalar(
                        a3[:], a2[:], tvp[:, 2:3], k2, op0=mult, op1=add
                    )
                    a4 = spool.tile([P, 1], f32, tag="a4")
                    nc.vector.tensor_scalar(
                        a4[:], a3[:], tvp[:, 3:4], k3, op0=mult, op1=add
                    )
                    # --- out = x * alpha on ScalarE ---
                    nc.scalar.activation(
                        o2[:, h, :], xt[:, :D], Copy, bias=0.0, scale=a4[:]
                    )
                else:
                    # --- vector path dots (accumulate 1 + t_i directly) ---
                    tv = spool.tile([P, L], f32, tag="tv")
                    for i in range(L):
                        if i < 2:
                            scr = scrpool.tile([P, DE], f32, tag="scr_v")
                            nc.vector.scalar_tensor_tensor(
                                out=scr[:], in0=xt, scalar=1.0,
                                in1=wb[:, i, :], op0=mult, op1=mult,
                                accum_out=tv[:, i : i + 1],
                            )
                        else:
                            scr = scrpool.tile([P, DE], f32, tag="scr_g")
                            with tc.high_priority(offset=40):
                                nc.gpsimd.tensor_tensor(
                                    out=scr[:], in0=xt, in1=wb[:, i, :],
                                    op=mult,
                                )
                            nc.scalar.activation(
                                scr[:], scr[:], Copy, bias=0.0, scale=1.0,
                                accum_out=tv[:, i : i + 1],
                            )
                    # --- recurrence on ScalarE via chained activations ---
                    a2 = spool.tile([P, 1], f32, tag="b2")
                    nc.scalar.activation(
                        a2[:], tv[:, 1:2], Copy, bias=k1, scale=tv[:, 0:1]
                    )
                    a3 = spool.tile([P, 1], f32, tag="b3")
                    nc.scalar.activation(
                        a3[:], tv[:, 2:3], Copy, bias=k2, scale=a2[:]
                    )
                    a4 = spool.tile([P, 1], f32, tag="b4")
                    nc.scalar.activation(
                        a4[:], tv[:, 3:4], Copy, bias=k3, scale=a3[:]
                    )
                    # --- out = x * alpha on DVE (single-src 2x mode) ---
                    nc.vector.tensor_scalar_mul(o2[:, h, :], xt[:, :D], a4[:])

                if s_idx == NT // 4 - 1:
                    # fine-grained last group: drain the tail sooner
                    nc.sync.dma_start(
                        out=outr[s_idx, :, h, :], in_=o2[:, h, :]
                    )
                elif h == 3:
                    nc.sync.dma_start(out=outr[s_idx], in_=o2[:])

    nc.compile()
    return nc


def _make_in_maps(x, W):
    """Per-core input maps; x and W must already be float32 C-contiguous."""
    x_ext = np.empty((BATCH, DE), dtype=np.float32)
    x_ext[:, :D] = x
    x_ext[:, D] = 1.0
    w_ext = np.empty((L, DE), dtype=np.float32)
    w_ext[:, :D] = W
    w_ext[:, D] = 1.0
    # wb: w rows replicated across the 128 partitions
    wb = np.ascontiguousarray(np.broadcast_to(w_ext[None, :, :], (P, L, DE)))
    # wt: w^T chunks, wt[p, c, i] = w[i, c*128+p]
    wt = np.ascontiguousarray(W.reshape(L, NCH, P).transpose(2, 1, 0))
    ident = np.eye(P, dtype=np.float32)
    return [
        {
            "x": x_ext[c * SHARD : (c + 1) * SHARD],
            "wbd": wb,
            "wtd": wt,
            "idd": ident,
        }
        for c in range(NCORES)
    ]


def kernel(x, cross_weights, cross_bias):
    from concourse.bass_utils import run_bass_kernel_spmd

    x = np.ascontiguousarray(np.asarray(x, dtype=np.float32))
    W = np.ascontiguousarray(np.asarray(cross_weights, dtype=np.float32))
    Bb = np.asarray(cross_bias, dtype=np.float32)
    assert x.shape == (BATCH, D) and W.shape == (L, D) and Bb.shape == (L, D)

    # host-side scalar constants k_i = C_i . w_i with C_i = sum_{j<i} b_j
    C = np.zeros(D, dtype=np.float32)
    ks = []
    for i in range(L):
        ks.append(float(C @ W[i]))
        C = C + Bb[i]
    # ks[0] == 0 always (C_0 = 0); bake the other three
    k1, k2, k3 = ks[1], ks[2], ks[3]

    key = (k1, k2, k3)
    nc = _build_cache.get(key)
    if nc is None:
        nc = _build_program(k1, k2, k3)
        _build_cache[key] = nc

    in_maps = _make_in_maps(x, W)
    res = run_bass_kernel_spmd(nc, in_maps, list(range(NCORES)))
    full = np.concatenate([res.results[c]["out"] for c in range(NCORES)], axis=0)
    full += C[None, :]  # C4 broadcast-add on host
    return full



# revision 2
# speedup vs baseline: 1.6624x; 1.6624x over previous
"""DCN cross-network forward on 8 Trainium2 NeuronCores — 16-bit pipeline.

Reference computation (LAYER_NUM=4, INPUT_DIM=1024, BATCH=16384):
    x0 = x
    for i in range(4):
        s  = xi @ w[i]                      # [B] per-row scalar
        xi = x0 * s[:, None] + b[i] + xi

Algebraic collapse: every layer adds a per-row multiple of x0 plus a
constant vector, so
    x_i = alpha_i * x0 + C_i,   C_i = sum_{j<i} b[j]
    t_i = x0 . w[i]             (per-row scalars, the only real compute)
    k_i = C_i . w[i]            (host-computable scalar constants)
    alpha_{i+1} = alpha_i * (1 + t_i) + k_i,  alpha_0 = 1
    out = alpha_4 * x0 + C_4
which reads x exactly once and writes out exactly once.  The rel-err
gate (2e-2) leaves room for a 16-bit device pipeline:
  - host casts x to fp16 (exact dot precision budget: ~5e-4 rel err)
  - device reads fp16, computes t_i, alpha (fp32), writes out = alpha*x
    in bf16 (alpha reaches ~2e7, fp16 out would overflow)
  - host upcasts to fp32 and adds C_4 (zero device time)
This halves HBM traffic vs fp32: ~4.2MB in + ~4.2MB out per core.

Engine split per 128-row tile (16 tiles/core, 4 groups of 4):
  - PE tiles: fp16 chunk transposes (is_transpose keeps fp16 in PSUM),
    ScalarE PSUM->SBUF copy, 8 accumulating dot matmuls vs w^T [128,4].
  - DVE tiles: 4 fused scalar_tensor_tensor dots (2x_1P 16-bit mode).
  - Recurrence: batched per group, 6 strided DVE ops on [128,4] slices
    (raw-t form; no ones-column so every slice stays 4B-aligned).
  - Scale out = alpha * x: DVE tensor_scalar (4x mode) / ScalarE split.

Sharding: data-parallel over batch; each of 8 cores gets [2048, 1024].
"""

import sys

import numpy as np

sys.path.insert(0, "/opt/trn_rl_repo")

BATCH = 16384
D = 1024
L = 4
NCORES = 8
SHARD = BATCH // NCORES  # 2048
P = 128
NT = SHARD // P          # 16 row-tiles per core
NCH = D // P             # 8 contraction chunks
NG = NT // 4             # 4 groups of 4 tiles

# tiles whose dots run on the TensorEngine (transpose+matmul); rest on DVE
PE_TILES = frozenset({0, 1, 3, 4, 6, 7, 9, 10, 12, 13, 15})
# tiles whose out-scale runs on ScalarE (rest on DVE)
SCALE_SCALAR = frozenset({1, 5, 9, 13})

_build_cache: dict = {}


def _build_program(k1: float, k2: float, k3: float):
    """Build (and compile) the SPMD Bass program for one core's shard."""
    import concourse.bacc as bacc
    import concourse.mybir as mybir
    import concourse.tile as tile
    f16 = mybir.dt.float16
    bf16 = mybir.dt.bfloat16
    f32 = mybir.dt.float32
    mult = mybir.AluOpType.mult
    add = mybir.AluOpType.add
    Copy = mybir.ActivationFunctionType.Copy

    nc = bacc.Bacc("TRN2", target_bir_lowering=False, debug=False)

    x = nc.dram_tensor("x", [SHARD, D], f16, kind="ExternalInput").ap()
    wbd = nc.dram_tensor("wbd", [P, L, D], f16, kind="ExternalInput").ap()
    wtd = nc.dram_tensor("wtd", [P, NCH, L], f16, kind="ExternalInput").ap()
    idd = nc.dram_tensor("idd", [P, P], f16, kind="ExternalInput").ap()
    out = nc.dram_tensor("out", [SHARD, D], bf16, kind="ExternalOutput").ap()

    with tile.TileContext(nc) as tc:
        with (
            tc.tile_pool(name="consts", bufs=1) as cpool,
            tc.tile_pool(name="xin", bufs=4) as xpool,
            tc.tile_pool(name="xtr", bufs=3) as xtpool,
            tc.tile_pool(name="scr", bufs=4) as scrpool,
            tc.tile_pool(name="small", bufs=8) as spool,
            tc.tile_pool(name="outp", bufs=3) as opool,
            tc.tile_pool(name="ps_tr", bufs=3, space="PSUM") as pst,
            tc.tile_pool(name="ps_t", bufs=4, space="PSUM") as psv,
        ):
            ident = cpool.tile([P, P], f16)
            with tc.high_priority():
                nc.sync.dma_start(out=ident[:], in_=idd)
            # w^T chunks for the PE dot path: wt_sb[p, c, i] = w[i, c*128+p]
            wt_sb = cpool.tile([P, NCH, L], f16)
            with tc.high_priority():
                nc.sync.dma_start(out=wt_sb[:], in_=wtd)
            # w rows replicated across partitions (pre-broadcast on host)
            wb = cpool.tile([P, L, D], f16)
            with tc.high_priority():
                nc.sync.dma_start(out=wb[:], in_=wbd)

            xr = x.rearrange("(s h p) d -> s p h d", p=P, h=4)
            outr = out.rearrange("(s h p) d -> s p h d", p=P, h=4)
            xt2 = None
            o2 = None
            tvg = None
            a4g = None
            for j in range(NT):
                s_idx, h = j // 4, j % 4
                if h == 0:
                    xt2 = xpool.tile([P, 4, D], f16, tag="x")
                    with tc.high_priority(offset=15):
                        if s_idx == 0:
                            # fine-grained first group: start compute sooner
                            for hh in range(4):
                                nc.sync.dma_start(
                                    out=xt2[:, hh, :], in_=xr[s_idx, :, hh, :]
                                )
                        else:
                            nc.sync.dma_start(out=xt2[:], in_=xr[s_idx])
                    o2 = opool.tile([P, 4, D], bf16, tag="o")
                    tvg = spool.tile([P, 4, L], f32, tag="tv")
                xt = xt2[:, h, :]

                if j in PE_TILES:
                    # --- PE path: fp16 transposes, dots vs w^T chunks ---
                    xtp = pst.tile([P, NCH, P], f16, tag="xtp")
                    for c in range(NCH):
                        nc.tensor.transpose(
                            xtp[:, c, :],
                            xt[:, c * P : (c + 1) * P],
                            ident[:],
                        )
                    xts = xtpool.tile([P, NCH, P], f16, tag="xts")
                    nc.scalar.copy(out=xts[:], in_=xtp[:])
                    tps = psv.tile([P, L], f32, tag="tps")
                    for c in range(NCH):
                        nc.tensor.matmul(
                            tps[:],
                            lhsT=xts[:, c, :],
                            rhs=wt_sb[:, c, :],
                            start=(c == 0),
                            stop=(c == NCH - 1),
                        )
                    nc.scalar.activation(
                        tvg[:, h, :], tps[:], Copy, bias=0.0, scale=1.0
                    )
                else:
                    # --- DVE path: fused multiply+accumulate dots ---
                    for i in range(L):
                        scr = scrpool.tile([P, D], f16, tag="scr_v")
                        nc.vector.scalar_tensor_tensor(
                            out=scr[:], in0=xt, scalar=1.0,
                            in1=wb[:, i, :], op0=mult, op1=mult,
                            accum_out=tvg[:, h, i : i + 1],
                        )

                if h == 3:
                    # --- batched recurrence for the group (raw-t form) ---
                    # alpha4 = ((((1+t0)(1+t1)+k1)(1+t2)+k2)(1+t3))+k3
                    t0 = tvg[:, :, 0]
                    t1 = tvg[:, :, 1]
                    t2 = tvg[:, :, 2]
                    t3 = tvg[:, :, 3]
                    u = spool.tile([P, 4], f32, tag="u")
                    nc.vector.scalar_tensor_tensor(
                        out=u[:], in0=t1, scalar=1.0, in1=t0, op0=add, op1=mult
                    )
                    al2 = spool.tile([P, 4], f32, tag="al2")
                    nc.vector.scalar_tensor_tensor(
                        out=al2[:], in0=u[:], scalar=1.0 + k1, in1=t1,
                        op0=add, op1=add,
                    )
                    w3 = spool.tile([P, 4], f32, tag="w3")
                    nc.vector.scalar_tensor_tensor(
                        out=w3[:], in0=t2, scalar=1.0, in1=al2[:],
                        op0=add, op1=mult,
                    )
                    z = spool.tile([P, 4], f32, tag="z")
                    nc.vector.scalar_tensor_tensor(
                        out=z[:], in0=t3, scalar=1.0, in1=w3[:],
                        op0=add, op1=mult,
                    )
                    y = spool.tile([P, 4], f32, tag="y")
                    nc.vector.scalar_tensor_tensor(
                        out=y[:], in0=t3, scalar=k2, in1=z[:],
                        op0=mult, op1=add,
                    )
                    a4g = spool.tile([P, 4], f32, tag="a4")
                    nc.vector.tensor_scalar(
                        a4g[:], y[:], k2 + k3, None, op0=add
                    )
                    # --- out = x * alpha ---
                    for hh in range(4):
                        jj = s_idx * 4 + hh
                        if jj in SCALE_SCALAR:
                            nc.scalar.activation(
                                o2[:, hh, :], xt2[:, hh, :], Copy,
                                bias=0.0, scale=a4g[:, hh : hh + 1],
                            )
                        else:
                            nc.vector.tensor_scalar_mul(
                                o2[:, hh, :], xt2[:, hh, :],
                                a4g[:, hh : hh + 1],
                            )
                    if s_idx == NG - 1:
                        # fine-grained last group: drain the tail sooner
                        for hh in range(4):
                            nc.sync.dma_start(
                                out=outr[s_idx, :, hh, :], in_=o2[:, hh, :]
                            )
                    else:
                        nc.sync.dma_start(out=outr[s_idx], in_=o2[:])

    nc.compile()
    return nc


def _make_in_maps(x16, W16):
    """Per-core input maps; x16/W16 are fp16 C-contiguous [B,D] and [L,D]."""
    # wb: w rows replicated across the 128 partitions
    wb = np.ascontiguousarray(np.broadcast_to(W16[None, :, :], (P, L, D)))
    # wt: w^T chunks, wt[p, c, i] = w[i, c*128+p]
    wt = np.ascontiguousarray(W16.reshape(L, NCH, P).transpose(2, 1, 0))
    ident = np.eye(P, dtype=np.float16)
    return [
        {
            "x": x16[c * SHARD : (c + 1) * SHARD],
            "wbd": wb,
            "wtd": wt,
            "idd": ident,
        }
        for c in range(NCORES)
    ]


def kernel(x, cross_weights, cross_bias):
    from concourse.bass_utils import run_bass_kernel_spmd

    x = np.asarray(x, dtype=np.float32)
    W = np.asarray(cross_weights, dtype=np.float32)
    Bb = np.asarray(cross_bias, dtype=np.float32)
    assert x.shape == (BATCH, D) and W.shape == (L, D) and Bb.shape == (L, D)

    # host-side scalar constants k_i = C_i . w_i with C_i = sum_{j<i} b_j
    C = np.zeros(D, dtype=np.float32)
    ks = []
    for i in range(L):
        ks.append(float(C @ W[i]))
        C = C + Bb[i]
    # ks[0] == 0 always (C_0 = 0); bake the other three
    k1, k2, k3 = ks[1], ks[2], ks[3]

    key = (k1, k2, k3)
    nc = _build_cache.get(key)
    if nc is None:
        nc = _build_program(k1, k2, k3)
        _build_cache[key] = nc

    x16 = np.ascontiguousarray(x.astype(np.float16))
    W16 = np.ascontiguousarray(W.astype(np.float16))
    in_maps = _make_in_maps(x16, W16)
    res = run_bass_kernel_spmd(nc, in_maps, list(range(NCORES)))
    full = np.concatenate(
        [np.asarray(res.results[c]["out"]) for c in range(NCORES)], axis=0
    ).astype(np.float32)
    full += C[None, :]  # C4 broadcast-add on host
    return full


# revision 3
# speedup vs baseline: 2.1640x; 1.3017x over previous
"""DCN cross-network forward on 8 Trainium2 NeuronCores — 16-bit pipeline.

Reference computation (LAYER_NUM=4, INPUT_DIM=1024, BATCH=16384):
    x0 = x
    for i in range(4):
        s  = xi @ w[i]                      # [B] per-row scalar
        xi = x0 * s[:, None] + b[i] + xi

Algebraic collapse: every layer adds a per-row multiple of x0 plus a
constant vector, so
    x_i = alpha_i * x0 + C_i,   C_i = sum_{j<i} b[j]
    t_i = x0 . w[i]             (per-row scalars, the only real compute)
    k_i = C_i . w[i]            (host-computable scalar constants)
    alpha_{i+1} = alpha_i * (1 + t_i) + k_i,  alpha_0 = 1
    out = alpha_4 * x0 + C_4
which reads x exactly once and writes out exactly once.  The rel-err
gate (2e-2) leaves room for a 16-bit device pipeline:
  - host casts x to fp16 (dot rel err ~5e-4)
  - device reads fp16, computes t_i and alpha (fp32), writes
    out = alpha*x in bf16 (alpha reaches ~2e7; fp16 out would overflow)
  - host upcasts to fp32 and adds C_4
This halves HBM traffic vs fp32: ~4.2MB in + ~4.2MB out per core — the
memory roofline is ~24us/core.

Device pipeline per 128-row tile (16 tiles/core, 4 groups of 4):
  - TensorE: 8 fp16 chunk transposes (is_transpose keeps fp16 in PSUM,
    ~107ns cadence), then 8 accumulating dot matmuls vs w^T [128,4]
    chunks (~26ns cadence, LDWEIGHTS hidden) -> t in PSUM [128,4] fp32.
  - ScalarE: PSUM->SBUF copy of the transposed chunks (dot matmul lhsT
    must live in SBUF).
  - DVE: batched per-group alpha recurrence (6 strided ops on [128,4])
    and the out-scale (tensor_scalar 16-bit fast mode).
Host-side layout shuffle gives every DMA 128 partitions x 8KB
contiguous descriptors; identity/w^T consts are issued first so the PE
can start at ~3us.

Sharding: data-parallel over batch; each of 8 cores gets [2048, 1024].
"""

import sys

import numpy as np

sys.path.insert(0, "/opt/trn_rl_repo")

BATCH = 16384
D = 1024
L = 4
NCORES = 8
SHARD = BATCH // NCORES  # 2048
P = 128
NT = SHARD // P          # 16 row-tiles per core
NCH = D // P             # 8 contraction chunks
NG = NT // 4             # 4 groups of 4 tiles

_build_cache: dict = {}


def _build_program(k1: float, k2: float, k3: float):
    """Build (and compile) the SPMD Bass program for one core's shard."""
    import concourse.bacc as bacc
    import concourse.mybir as mybir
    import concourse.tile as tile
    f16 = mybir.dt.float16
    bf16 = mybir.dt.bfloat16
    f32 = mybir.dt.float32
    mult = mybir.AluOpType.mult
    add = mybir.AluOpType.add

    nc = bacc.Bacc("TRN2", target_bir_lowering=False, debug=False)

    # host pre-shuffled layout: x[s, p, h, :] = row (s*512 + h*128 + p)
    x = nc.dram_tensor("x", [NG, P, 4, D], f16, kind="ExternalInput").ap()
    wtd = nc.dram_tensor("wtd", [P, NCH, L], f16, kind="ExternalInput").ap()
    idd = nc.dram_tensor("idd", [P, P], f16, kind="ExternalInput").ap()
    out = nc.dram_tensor("out", [NG, P, 4, D], bf16, kind="ExternalOutput").ap()

    with tile.TileContext(nc) as tc:
        with (
            tc.tile_pool(name="consts", bufs=1) as cpool,
            tc.tile_pool(name="xin", bufs=4) as xpool,
            tc.tile_pool(name="xtr", bufs=3) as xtpool,
            tc.tile_pool(name="small", bufs=8) as spool,
            tc.tile_pool(name="outp", bufs=3) as opool,
            tc.tile_pool(name="ps_tr", bufs=3, space="PSUM") as pst,
            tc.tile_pool(name="ps_t", bufs=2, space="PSUM") as psv,
        ):
            ident = cpool.tile([P, P], f16)
            wt_sb = cpool.tile([P, NCH, L], f16)
            with tc.high_priority(offset=1000):
                nc.sync.dma_start(out=ident[:], in_=idd)
                nc.sync.dma_start(out=wt_sb[:], in_=wtd)

            xt2 = None
            o2 = None
            tps = None
            tvg = None
            for j in range(NT):
                s_idx, h = j // 4, j % 4
                if h == 0:
                    xt2 = xpool.tile([P, 4, D], f16, tag="x")
                    with tc.high_priority(offset=15):
                        if s_idx == 0:
                            # fine-grained first group: start compute sooner
                            for hh in range(4):
                                nc.sync.dma_start(
                                    out=xt2[:, hh, :], in_=x[s_idx, :, hh, :]
                                )
                        else:
                            nc.sync.dma_start(out=xt2[:], in_=x[s_idx])
                    o2 = opool.tile([P, 4, D], bf16, tag="o")
                    tps = psv.tile([P, 4, L], f32, tag="tps")
                xt = xt2[:, h, :]

                # --- TensorE: transpose chunks, then dot vs w^T chunks ---
                xtp = pst.tile([P, NCH, P], f16, tag="xtp")
                for c in range(NCH):
                    nc.tensor.transpose(
                        xtp[:, c, :], xt[:, c * P : (c + 1) * P], ident[:]
                    )
                xts = xtpool.tile([P, NCH, P], f16, tag="xts")
                nc.scalar.copy(out=xts[:], in_=xtp[:])
                for c in range(NCH):
                    nc.tensor.matmul(
                        tps[:, h, :],
                        lhsT=xts[:, c, :],
                        rhs=wt_sb[:, c, :],
                        start=(c == 0),
                        stop=(c == NCH - 1),
                    )

                if h == 3:
                    # --- batched recurrence for the group (raw-t form) ---
                    # alpha4 = ((((1+t0)(1+t1)+k1)(1+t2)+k2)(1+t3))+k3
                    tvg = spool.tile([P, 4, L], f32, tag="tv")
                    nc.vector.tensor_copy(tvg[:], tps[:])
                    t0 = tvg[:, :, 0]
                    t1 = tvg[:, :, 1]
                    t2 = tvg[:, :, 2]
                    t3 = tvg[:, :, 3]
                    u = spool.tile([P, 4], f32, tag="u")
                    nc.vector.scalar_tensor_tensor(
                        out=u[:], in0=t1, scalar=1.0, in1=t0, op0=add, op1=mult
                    )
                    al2 = spool.tile([P, 4], f32, tag="al2")
                    nc.vector.scalar_tensor_tensor(
                        out=al2[:], in0=u[:], scalar=1.0 + k1, in1=t1,
                        op0=add, op1=add,
                    )
                    w3 = spool.tile([P, 4], f32, tag="w3")
                    nc.vector.scalar_tensor_tensor(
                        out=w3[:], in0=t2, scalar=1.0, in1=al2[:],
                        op0=add, op1=mult,
                    )
                    z = spool.tile([P, 4], f32, tag="z")
                    nc.vector.scalar_tensor_tensor(
                        out=z[:], in0=t3, scalar=1.0, in1=w3[:],
                        op0=add, op1=mult,
                    )
                    y = spool.tile([P, 4], f32, tag="y")
                    nc.vector.scalar_tensor_tensor(
                        out=y[:], in0=t3, scalar=k2, in1=z[:],
                        op0=mult, op1=add,
                    )
                    a4g = spool.tile([P, 4], f32, tag="a4")
                    nc.vector.tensor_scalar(
                        a4g[:], y[:], k2 + k3, None, op0=add
                    )
                    # --- out = x * alpha (DVE 16-bit fast mode) ---
                    for hh in range(4):
                        nc.vector.tensor_scalar_mul(
                            o2[:, hh, :], xt2[:, hh, :], a4g[:, hh : hh + 1]
                        )
                    if s_idx == NG - 1:
                        # fine-grained last group: drain the tail sooner
                        for hh in range(4):
                            nc.sync.dma_start(
                                out=out[s_idx, :, hh, :], in_=o2[:, hh, :]
                            )
                    else:
                        nc.sync.dma_start(out=out[s_idx], in_=o2[:])

    nc.compile()
    return nc


def _shuffle(x16):
    """[2048, 1024] -> [NG, P, 4, D] with x'[s, p, h] = x[s*512 + h*128 + p]."""
    return np.ascontiguousarray(
        x16.reshape(NG, 4, P, D).transpose(0, 2, 1, 3)
    )


def _make_in_maps(x16, W16):
    """Per-core input maps; x16/W16 are fp16 C-contiguous [B,D] and [L,D]."""
    # wt: w^T chunks, wt[p, c, i] = w[i, c*128+p]
    wt = np.ascontiguousarray(W16.reshape(L, NCH, P).transpose(2, 1, 0))
    ident = np.eye(P, dtype=np.float16)
    return [
        {
            "x": _shuffle(x16[c * SHARD : (c + 1) * SHARD]),
            "wtd": wt,
            "idd": ident,
        }
        for c in range(NCORES)
    ]


def kernel(x, cross_weights, cross_bias):
    from concourse.bass_utils import run_bass_kernel_spmd

    x = np.asarray(x, dtype=np.float32)
    W = np.asarray(cross_weights, dtype=np.float32)
    Bb = np.asarray(cross_bias, dtype=np.float32)
    assert x.shape == (BATCH, D) and W.shape == (L, D) and Bb.shape == (L, D)

    # host-side scalar constants k_i = C_i . w_i with C_i = sum_{j<i} b_j
    C = np.zeros(D, dtype=np.float32)
    ks = []
    for i in range(L):
        ks.append(float(C @ W[i]))
        C = C + Bb[i]
    # ks[0] == 0 always (C_0 = 0); bake the other three
    k1, k2, k3 = ks[1], ks[2], ks[3]

    key = (k1, k2, k3)
    nc = _build_cache.get(key)
    if nc is None:
        nc = _build_program(k1, k2, k3)
        _build_cache[key] = nc

    x16 = np.ascontiguousarray(x.astype(np.float16))
    W16 = np.ascontiguousarray(W.astype(np.float16))
    in_maps = _make_in_maps(x16, W16)
    res = run_bass_kernel_spmd(nc, in_maps, list(range(NCORES)))
    # un-shuffle: out'[s, p, h] -> row (s*512 + h*128 + p), upcast, add C4
    full = np.empty((BATCH, D), dtype=np.float32)
    for c in range(NCORES):
        oc = np.asarray(res.results[c]["out"])  # [NG, P, 4, D] bf16
        full[c * SHARD : (c + 1) * SHARD] = (
            oc.transpose(0, 2, 1, 3).reshape(SHARD, D).astype(np.float32)
        )
    full += C[None, :]  # C4 broadcast-add on host
    return full
